# revision 14
# baseline (speedup 1.0000x reference)
"""BatchHardTripletLoss on 8 Trainium2 NeuronCores — flipped + norm-dealt.

Layout: rows label-sorted on host; each core streams its 1024 anchors
(free dim) against all B=8192 embeddings as 64 column chunks of 128
(partition dim).

  - Own chunks (K, exact): the ~10 chunks holding the core's own labels
    (plus fillers). PE adds a one-hot penalty matmul (+1024 on same-label);
    ACT copies psum->f16 with the exact per-partition sq_j bias fused
    (Identity activation, [128,1] bias AP); DVE runs same-buffer min
    (hardest-neg) and max (hardest-pos) chains.
  - Dealt chunks (64-K, approx): remaining columns are sorted by ||x||^2
    and dealt so each partition holds consecutive-rank norms across all
    chunks -> sq_j is near-constant per partition. Chunk pairs share one
    4-bank psum tile; most pairs go ACT Copy + f16 same-tile fold, a few
    go via a strided DVE pair-min reduce straight from psum. A binary
    fold tree (f16 2x, same-tile halves) collapses the rest; the single
    per-partition bias s_hat is fused into the final merge STT.
  - Partition-direction reductions run on the otherwise-idle GPSIMD via
    partition_all_reduce(max) (hn negated first); sqrt/relu/mean on host.
"""

import sys

import numpy as np

if "/opt/trn_rl_repo" not in sys.path:
    sys.path.insert(0, "/opt/trn_rl_repo")

from concourse import bacc, bass, bass_isa, mybir, tile
from concourse.bass_utils import run_bass_kernel_spmd

B = 8192
D = 128
C = 128
N_CORES = 8
R = B // N_CORES          # anchors per core
NCH = B // 128            # column chunks (64)
PEN = 1024.0
ACC_INIT = 60000.0

F16 = mybir.dt.float16
F32 = mybir.dt.float32
ALU = mybir.AluOpType
ACTF = mybir.ActivationFunctionType

_NC_CACHE = {}


def _build_nc(kown):
    npair = (NCH - kown) // 2
    dve_pairs = {8, 17, npair - 1}
    nc = bacc.Bacc(None, target_bir_lowering=False)

    xt_d = nc.declare_dram_parameter("xt", [128, B], F16, isOutput=False)
    xls_d = nc.declare_dram_parameter("xls", [128, R], F16, isOutput=False)
    ohs_d = nc.declare_dram_parameter("ohs", [128, R], F16, isOutput=False)
    ohk_d = nc.declare_dram_parameter("ohk", [128, kown * 128], F16, isOutput=False)
    sqc_d = nc.declare_dram_parameter("sqc", [128, kown], F32, isOutput=False)
    shat_d = nc.declare_dram_parameter("shat", [128, 1], F32, isOutput=False)
    hn1_d = nc.declare_dram_parameter("hn1", [1, R], F32, isOutput=True)
    hp1_d = nc.declare_dram_parameter("hp1", [1, R], F32, isOutput=True)

    with tile.TileContext(nc) as tc:
        with tc.tile_pool(name="const", bufs=1) as cp:
            XTS = [cp.tile([128, 1024], F16, name=f"xts{s}") for s in range(8)]
            XLS = cp.tile([128, R], F16)
            OHS = cp.tile([128, R], F16)
            OHK = cp.tile([128, kown * 128], F16)
            SQC = cp.tile([128, kown], F32)
            SHAT = cp.tile([128, 1], F32)
            ACCF = cp.tile([128, R], F16)
            NACC = cp.tile([128, R], F16)
            QF = cp.tile([128, 3072], F16, name="qf")
            PMAX = cp.tile([128, R], F32)
            NMAX = cp.tile([128, R], F32)

            nc.sync.dma_start(XLS[:], xls_d[:])
            nc.sync.dma_start(XTS[0][:], xt_d[:, 0:1024])
            nc.sync.dma_start(SQC[:], sqc_d[:])
            nc.sync.dma_start(OHK[:], ohk_d[:])
            nc.sync.dma_start(OHS[:], ohs_d[:])
            nc.sync.dma_start(XTS[1][:], xt_d[:, 1024:2048])
            for s in range(2, 8):
                nc.sync.dma_start(XTS[s][:], xt_d[:, s * 1024 : (s + 1) * 1024])
            nc.sync.dma_start(SHAT[:], shat_d[:])

            def chunk_lhs(ch):
                return XTS[ch // 8][:, (ch % 8) * 128 : (ch % 8) * 128 + 128]

            # ---------------- phase 1: own chunks (exact) ----------------
            # Q buffers: [0:1024] T-slot, [1024:2048] ACC-slot, [2048:3072]
            # ACP-slot; same-tile input pairs keep DVE TTs on the fast path.
            with (
                tc.tile_pool(name="opsum", bufs=4, space=bass.MemorySpace.PSUM) as po,
                tc.tile_pool(name="qpool", bufs=2) as qp,
            ):
                Qprev = qp.tile([128, 3072], F16, tag="q")
                nc.vector.memset(Qprev[:, 1024:2048], ACC_INIT)
                nc.vector.memset(Qprev[:, 2048:3072], -ACC_INIT)
                for c in range(kown):
                    ps = po.tile([128, 1024], F32, tag="ps")
                    lhs = chunk_lhs(c)
                    for h in range(2):
                        sl = slice(h * 512, (h + 1) * 512)
                        nc.tensor.matmul(ps[:, sl], lhs, XLS[:, sl],
                                         start=True, stop=False)
                        nc.tensor.matmul(ps[:, sl],
                                         OHK[:, c * 128 : (c + 1) * 128],
                                         OHS[:, sl], start=False, stop=True)
                    Q = QF if c == kown - 1 else qp.tile([128, 3072], F16, tag="q")
                    nc.scalar.activation(Qprev[:, 0:1024], ps[:], ACTF.Identity,
                                         bias=SQC[:, c : c + 1])
                    nc.vector.tensor_tensor(Q[:, 1024:2048], Qprev[:, 0:1024],
                                            Qprev[:, 1024:2048], op=ALU.min)
                    nc.vector.tensor_tensor(Q[:, 2048:3072], Qprev[:, 0:1024],
                                            Qprev[:, 2048:3072], op=ALU.max)
                    Qprev = Q
                # hp partition-max on idle gpsimd, early (ACP final already)
                nc.gpsimd.partition_all_reduce(
                    PMAX[:], Qprev[:, 2048:3072], channels=128,
                    reduce_op=bass_isa.ReduceOp.max,
                )
                nc.sync.dma_start(hp1_d[:], PMAX[0:1, :])

            # ---------------- phase 2: dealt pairs (fold tree) -----------
            levels = {}
            pend = []

            def tree_push(level, write_fn, tp):
                buf = levels.get(level)
                if buf is None:
                    nb = tp.tile([128, 2048], F16, tag=f"tr{level}")
                    write_fn(nb[:, 0:1024])
                    levels[level] = nb
                else:
                    write_fn(buf[:, 1024:2048])
                    levels[level] = None
                    tree_push(
                        level + 1,
                        lambda dst, b=buf: nc.vector.tensor_tensor(
                            dst, b[:, 0:1024], b[:, 1024:2048], op=ALU.min),
                        tp,
                    )

            with (
                tc.tile_pool(name="dpsum", bufs=2, space=bass.MemorySpace.PSUM) as pd,
                tc.tile_pool(name="gpool", bufs=3) as gp,
                tc.tile_pool(name="tpool", bufs=2) as tp,
            ):
                for t in range(npair):
                    ps = pd.tile([128, 2048], F32, tag="ps")
                    for j in range(2):
                        lhs = chunk_lhs(kown + 2 * t + j)
                        for h in range(2):
                            sl = slice(j * 1024 + h * 512, j * 1024 + (h + 1) * 512)
                            nc.tensor.matmul(ps[:, sl], lhs,
                                             XLS[:, h * 512 : (h + 1) * 512],
                                             start=True, stop=True)
                    if t in dve_pairs:
                        # DVE path: strided pair-min reduce straight from psum
                        tree_push(0, lambda dst, p=ps: nc.vector.tensor_reduce(
                            dst, p[:].rearrange("p (c i) -> p i c", c=2),
                            axis=mybir.AxisListType.X, op=ALU.min), tp)
                    else:
                        # ACT path: plain psum->f16 copy, sbuf fold at 2x
                        G = gp.tile([128, 2048], F16, tag="g")
                        nc.scalar.activation(G[:], ps[:], ACTF.Copy)
                        tree_push(0, lambda dst, g=G: nc.vector.tensor_tensor(
                            dst, g[:, 0:1024], g[:, 1024:2048], op=ALU.min), tp)

                # collapse leftover half-filled tree buffers (two-tile TTs)
                for lv in sorted(levels):
                    if levels[lv] is not None:
                        pend.append(levels[lv][:, 0:1024])
                nx = 0
                while len(pend) > 1:
                    a = pend.pop(0)
                    b = pend.pop(0)
                    nb = tp.tile([128, 1024], F16, tag=f"trx{nx}")
                    nx += 1
                    nc.vector.tensor_tensor(nb[:], a, b, op=ALU.min)
                    pend.append(nb[:])
                # bias the dealt result and merge with the exact own chain in
                # one fused op: ACCF = (dealt_min + s_hat) min ACC_own
                nc.vector.scalar_tensor_tensor(
                    ACCF[:], pend[0], SHAT[:, 0:1], QF[:, 1024:2048],
                    op0=ALU.add, op1=ALU.min,
                )
                # negate so the gpsimd all-reduce (max-only) computes the min
                nc.vector.tensor_scalar_mul(NACC[:], ACCF[:], -1.0)
                nc.gpsimd.partition_all_reduce(
                    NMAX[:], NACC[:], channels=128,
                    reduce_op=bass_isa.ReduceOp.max,
                )

            nc.sync.dma_start(hn1_d[:], NMAX[0:1, :])

    nc.compile()
    return nc


def _get_nc(kown):
    if kown not in _NC_CACHE:
        _NC_CACHE[kown] = _build_nc(kown)
    return _NC_CACHE[kown]


def _prep_in_maps(embeddings, labels):
    x = np.asarray(embeddings, dtype=np.float32)
    lab = np.asarray(labels).astype(np.int64)
    order = np.argsort(lab, kind="stable")
    lab_s = lab[order]
    xs = x[order]
    xt = np.ascontiguousarray(xs.T).astype(np.float16)   # [128, B]
    sq = (xs.astype(np.float64) ** 2).sum(1).astype(np.float32)
    own_sets = []
    K = 0
    for m in range(N_CORES):
        mylab = lab_s[m * R : (m + 1) * R]
        own_idx = np.flatnonzero((lab_s >= mylab.min()) & (lab_s <= mylab.max()))
        own_sets.append(own_idx)
        K = max(K, -(-len(own_idx) // 128))
    K += K % 2  # keep the dealt chunk count even
    in_maps = []
    for m in range(N_CORES):
        own_idx = own_sets[m]
        mask = np.zeros(B, bool)
        mask[own_idx] = True
        non_own = np.flatnonzero(~mask)
        n_fill = K * 128 - len(own_idx)
        fill, dealt = non_own[:n_fill], non_own[n_fill:]
        own_cols = np.concatenate([own_idx, fill])
        dsort = dealt[np.argsort(sq[dealt], kind="stable")]
        deal_mat = dsort.reshape(128, NCH - K)           # [partition, chunk]
        cols = np.concatenate([own_cols, deal_mat.T.reshape(-1)])
        mylab = lab_s[m * R : (m + 1) * R]
        in_maps.append({
            "xt": np.ascontiguousarray(xt[:, cols]),
            "xls": np.ascontiguousarray(
                (-2.0 * xs[m * R : (m + 1) * R].T)).astype(np.float16),
            "ohs": (PEN * (mylab[None, :] == np.arange(C)[:, None])).astype(np.float16),
            "ohk": (lab_s[own_cols][None, :] == np.arange(C)[:, None]).astype(np.float16),
            "sqc": np.ascontiguousarray(sq[own_cols].reshape(K, 128).T),
            "shat": sq[deal_mat].mean(1, dtype=np.float64).astype(np.float32).reshape(128, 1),
        })
    return in_maps, lab, order, sq, K


def run_cores(embeddings, labels, trace=False, **kw):
    in_maps, lab, order, sq, K = _prep_in_maps(embeddings, labels)
    nc = _get_nc(K)
    res = run_bass_kernel_spmd(nc, in_maps, list(range(N_CORES)), trace=trace, **kw)
    hn2 = np.concatenate(
        [-np.asarray(r["hn1"], np.float32).reshape(R) for r in res.results]
    )
    hp2 = np.concatenate(
        [np.asarray(r["hp1"], np.float32).reshape(R) for r in res.results]
    )
    hn = np.sqrt(np.maximum(hn2 + sq, 0.0))
    hp = np.sqrt(np.maximum(hp2 + sq - PEN, 0.0))
    pr_sorted = np.maximum(hp - hn + 1.0, 0.0)
    pr = np.empty(B, np.float32)
    pr[order] = pr_sorted
    counts = np.bincount(lab, minlength=C)
    valid = (counts[lab] >= 2) & (counts[lab] <= B - 1)
    nv = int(valid.sum())
    loss = float((pr * valid).sum() / nv) if nv > 0 else 0.0
    return np.float32(loss), res


def kernel(embeddings, labels):
    loss, _ = run_cores(embeddings, labels, trace=False)
    return loss


# revision 16
# speedup vs baseline: 1.0616x; 1.0616x over previous
"""BatchHardTripletLoss on 8 Trainium2 NeuronCores — flipped + norm-dealt.

Layout: rows label-sorted on host; each core streams its 1024 anchors
(free dim) against all B=8192 embeddings as 64 column chunks of 128
(partition dim).

  - Own chunks (K, exact): the ~10 chunks holding the core's own labels
    (plus fillers). PE adds a one-hot penalty matmul (+1024 on same-label);
    ACT copies psum->f16 with the exact per-partition sq_j bias fused
    (Identity activation, [128,1] bias AP); DVE runs same-buffer min
    (hardest-neg) and max (hardest-pos) chains. Scheduled between early
    dealt pairs so the one-hot/sq DMAs hide behind compute.
  - Dealt chunks (64-K, approx): remaining columns are sorted by ||x||^2
    and dealt so each partition holds consecutive-rank norms across all
    chunks -> sq_j is near-constant per partition. Chunk pairs share one
    4-bank psum tile; most pairs go ACT Copy + f16 same-tile fold, a few
    go via a strided DVE pair-min reduce straight from psum. A binary
    fold tree (f16 2x) collapses the rest; the per-partition bias s_hat
    is fused into the final merge STT.
  - hardest-pos partition-max runs on the idle GPSIMD (overlapped);
    hardest-neg finishes with PE transposes + DVE free-dim reduces.
    sqrt/relu/mean finalize on host.
"""

import sys

import numpy as np

if "/opt/trn_rl_repo" not in sys.path:
    sys.path.insert(0, "/opt/trn_rl_repo")

from concourse import bacc, bass, bass_isa, mybir, tile
from concourse.bass_utils import run_bass_kernel_spmd

B = 8192
D = 128
C = 128
N_CORES = 8
R = B // N_CORES          # anchors per core
NCH = B // 128            # column chunks (64)
RT = R // 128             # anchor blocks for the tail transposes (8)
PEN = 1024.0
ACC_INIT = 60000.0

F16 = mybir.dt.float16
F32 = mybir.dt.float32
ALU = mybir.AluOpType
ACTF = mybir.ActivationFunctionType

_NC_CACHE = {}


def _build_nc(kown):
    npair = (NCH - kown) // 2
    nown = kown // 2
    dve_pairs = {8, 17, npair - 1}
    nc = bacc.Bacc(None, target_bir_lowering=False)

    xt_d = nc.declare_dram_parameter("xt", [128, B], F16, isOutput=False)
    xls_d = nc.declare_dram_parameter("xls", [128, R], F16, isOutput=False)
    ohs_d = nc.declare_dram_parameter("ohs", [128, R], F16, isOutput=False)
    ohk_d = nc.declare_dram_parameter("ohk", [128, kown * 128], F16, isOutput=False)
    sqc_d = nc.declare_dram_parameter("sqc", [128, kown], F32, isOutput=False)
    shat_d = nc.declare_dram_parameter("shat", [128, 1], F32, isOutput=False)
    idn_d = nc.declare_dram_parameter("idn", [128, 128], F16, isOutput=False)
    hn2_d = nc.declare_dram_parameter("hn2", [128, RT], F32, isOutput=True)
    hp1_d = nc.declare_dram_parameter("hp1", [1, R], F32, isOutput=True)

    with tile.TileContext(nc) as tc:
        with tc.tile_pool(name="const", bufs=1) as cp:
            XTS = [cp.tile([128, 1024], F16, name=f"xts{s}") for s in range(8)]
            XLS = cp.tile([128, R], F16)
            OHS = cp.tile([128, R], F16)
            OHK = cp.tile([128, kown * 128], F16)
            SQC = cp.tile([128, kown], F32)
            SHAT = cp.tile([128, 1], F32)
            IDN = cp.tile([128, 128], F16)
            ACCF = cp.tile([128, R], F16)
            QF = cp.tile([128, 3072], F16, name="qf")
            PMAX = cp.tile([128, R], F32)
            HN2 = cp.tile([128, RT], F32)

            nc.sync.dma_start(XLS[:], xls_d[:])
            nc.sync.dma_start(XTS[1][:], xt_d[:, 1024:2048])
            nc.sync.dma_start(XTS[2][:], xt_d[:, 2048:3072])
            nc.sync.dma_start(OHK[:], ohk_d[:])
            nc.sync.dma_start(OHS[:], ohs_d[:])
            nc.sync.dma_start(SQC[:], sqc_d[:])
            nc.sync.dma_start(XTS[0][:], xt_d[:, 0:1024])
            for s in range(3, 8):
                nc.sync.dma_start(XTS[s][:], xt_d[:, s * 1024 : (s + 1) * 1024])
            nc.sync.dma_start(SHAT[:], shat_d[:])
            nc.sync.dma_start(IDN[:], idn_d[:])

            def chunk_lhs(ch):
                return XTS[ch // 8][:, (ch % 8) * 128 : (ch % 8) * 128 + 128]

            levels = {}
            pend = []

            def tree_push(level, write_fn, tp):
                buf = levels.get(level)
                if buf is None:
                    nb = tp.tile([128, 2048], F16, tag=f"tr{level}")
                    write_fn(nb[:, 0:1024])
                    levels[level] = nb
                else:
                    write_fn(buf[:, 1024:2048])
                    levels[level] = None
                    tree_push(
                        level + 1,
                        lambda dst, b=buf: nc.vector.tensor_tensor(
                            dst, b[:, 0:1024], b[:, 1024:2048], op=ALU.min),
                        tp,
                    )

            # interleave: 2 dealt pairs first (they only need XLS+XT1), then
            # the own pairs (their one-hot/sq DMAs have landed by then), then
            # the remaining dealt pairs.
            sched = [("d", 0), ("d", 1)]
            sched += [("o", u) for u in range(nown)]
            sched += [("d", t) for t in range(2, npair)]

            with (
                tc.tile_pool(name="dpsum", bufs=2, space=bass.MemorySpace.PSUM) as pd,
                tc.tile_pool(name="qpool", bufs=2) as qp,
                tc.tile_pool(name="gpool", bufs=3) as gp,
                tc.tile_pool(name="tpool", bufs=2) as tp,
            ):
                Qprev = qp.tile([128, 3072], F16, tag="q")
                nc.vector.memset(Qprev[:, 1024:2048], ACC_INIT)
                nc.vector.memset(Qprev[:, 2048:3072], -ACC_INIT)
                for kind, t in sched:
                    ps = pd.tile([128, 2048], F32, tag="ps")
                    if kind == "o":
                        for j in range(2):
                            c = 2 * t + j
                            lhs = chunk_lhs(c)
                            for h in range(2):
                                sl = slice(j * 1024 + h * 512,
                                           j * 1024 + (h + 1) * 512)
                                nc.tensor.matmul(ps[:, sl], lhs,
                                                 XLS[:, h * 512 : (h + 1) * 512],
                                                 start=True, stop=False)
                                nc.tensor.matmul(ps[:, sl],
                                                 OHK[:, c * 128 : (c + 1) * 128],
                                                 OHS[:, h * 512 : (h + 1) * 512],
                                                 start=False, stop=True)
                        for j in range(2):
                            c = 2 * t + j
                            Q = QF if c == kown - 1 else qp.tile(
                                [128, 3072], F16, tag="q")
                            nc.scalar.activation(
                                Qprev[:, 0:1024], ps[:, j * 1024 : (j + 1) * 1024],
                                ACTF.Identity, bias=SQC[:, c : c + 1])
                            nc.vector.tensor_tensor(
                                Q[:, 1024:2048], Qprev[:, 0:1024],
                                Qprev[:, 1024:2048], op=ALU.min)
                            nc.vector.tensor_tensor(
                                Q[:, 2048:3072], Qprev[:, 0:1024],
                                Qprev[:, 2048:3072], op=ALU.max)
                            Qprev = Q
                            if c == kown - 1:
                                # hp partition-max on the idle gpsimd, early
                                nc.gpsimd.partition_all_reduce(
                                    PMAX[:], QF[:, 2048:3072], channels=128,
                                    reduce_op=bass_isa.ReduceOp.max)
                                nc.sync.dma_start(hp1_d[:], PMAX[0:1, :])
                        continue
                    for j in range(2):
                        lhs = chunk_lhs(kown + 2 * t + j)
                        for h in range(2):
                            sl = slice(j * 1024 + h * 512, j * 1024 + (h + 1) * 512)
                            nc.tensor.matmul(ps[:, sl], lhs,
                                             XLS[:, h * 512 : (h + 1) * 512],
                                             start=True, stop=True)
                    if t in dve_pairs:
                        tree_push(0, lambda dst, p=ps: nc.vector.tensor_reduce(
                            dst, p[:].rearrange("p (c i) -> p i c", c=2),
                            axis=mybir.AxisListType.X, op=ALU.min), tp)
                    else:
                        G = gp.tile([128, 2048], F16, tag="g")
                        nc.scalar.activation(G[:], ps[:], ACTF.Copy)
                        tree_push(0, lambda dst, g=G: nc.vector.tensor_tensor(
                            dst, g[:, 0:1024], g[:, 1024:2048], op=ALU.min), tp)

                for lv in sorted(levels):
                    if levels[lv] is not None:
                        pend.append(levels[lv][:, 0:1024])
                nx = 0
                while len(pend) > 1:
                    a = pend.pop(0)
                    b = pend.pop(0)
                    nb = tp.tile([128, 1024], F16, tag=f"trx{nx}")
                    nx += 1
                    nc.vector.tensor_tensor(nb[:], a, b, op=ALU.min)
                    pend.append(nb[:])
                nc.vector.scalar_tensor_tensor(
                    ACCF[:], pend[0], SHAT[:, 0:1], QF[:, 1024:2048],
                    op0=ALU.add, op1=ALU.min,
                )

            with tc.tile_pool(name="fpsum", bufs=4, space=bass.MemorySpace.PSUM) as pf:
                for t in range(RT):
                    pn = pf.tile([128, 128], F16, tag="pn")
                    nc.tensor.transpose(pn[:], ACCF[:, t * 128 : (t + 1) * 128], IDN[:])
                    nc.vector.tensor_reduce(HN2[:, t : t + 1], pn[:],
                                            axis=mybir.AxisListType.X, op=ALU.min)

            nc.sync.dma_start(hn2_d[:], HN2[:])

    nc.compile()
    return nc


def _get_nc(kown):
    if kown not in _NC_CACHE:
        _NC_CACHE[kown] = _build_nc(kown)
    return _NC_CACHE[kown]


def _prep_in_maps(embeddings, labels):
    x = np.asarray(embeddings, dtype=np.float32)
    lab = np.asarray(labels).astype(np.int64)
    order = np.argsort(lab, kind="stable")
    lab_s = lab[order]
    xs = x[order]
    xt = np.ascontiguousarray(xs.T).astype(np.float16)   # [128, B]
    sq = (xs.astype(np.float64) ** 2).sum(1).astype(np.float32)
    idn = np.eye(128, dtype=np.float16)
    own_sets = []
    K = 0
    for m in range(N_CORES):
        mylab = lab_s[m * R : (m + 1) * R]
        own_idx = np.flatnonzero((lab_s >= mylab.min()) & (lab_s <= mylab.max()))
        own_sets.append(own_idx)
        K = max(K, -(-len(own_idx) // 128))
    K += K % 2  # keep chunk counts even (own pairs + dealt pairs)
    in_maps = []
    for m in range(N_CORES):
        own_idx = own_sets[m]
        mask = np.zeros(B, bool)
        mask[own_idx] = True
        non_own = np.flatnonzero(~mask)
        n_fill = K * 128 - len(own_idx)
        fill, dealt = non_own[:n_fill], non_own[n_fill:]
        own_cols = np.concatenate([own_idx, fill])
        dsort = dealt[np.argsort(sq[dealt], kind="stable")]
        deal_mat = dsort.reshape(128, NCH - K)           # [partition, chunk]
        cols = np.concatenate([own_cols, deal_mat.T.reshape(-1)])
        mylab = lab_s[m * R : (m + 1) * R]
        in_maps.append({
            "xt": np.ascontiguousarray(xt[:, cols]),
            "xls": np.ascontiguousarray(
                (-2.0 * xs[m * R : (m + 1) * R].T)).astype(np.float16),
            "ohs": (PEN * (mylab[None, :] == np.arange(C)[:, None])).astype(np.float16),
            "ohk": (lab_s[own_cols][None, :] == np.arange(C)[:, None]).astype(np.float16),
            "sqc": np.ascontiguousarray(sq[own_cols].reshape(K, 128).T),
            "shat": sq[deal_mat].mean(1, dtype=np.float64).astype(np.float32).reshape(128, 1),
            "idn": idn,
        })
    return in_maps, lab, order, sq, K


def run_cores(embeddings, labels, trace=False, **kw):
    in_maps, lab, order, sq, K = _prep_in_maps(embeddings, labels)
    nc = _get_nc(K)
    res = run_bass_kernel_spmd(nc, in_maps, list(range(N_CORES)), trace=trace, **kw)
    hn2 = np.concatenate(
        [np.asarray(r["hn2"], np.float32).T.reshape(R) for r in res.results]
    )
    hp2 = np.concatenate(
        [np.asarray(r["hp1"], np.float32).reshape(R) for r in res.results]
    )
    hn = np.sqrt(np.maximum(hn2 + sq, 0.0))
    hp = np.sqrt(np.maximum(hp2 + sq - PEN, 0.0))
    pr_sorted = np.maximum(hp - hn + 1.0, 0.0)
    pr = np.empty(B, np.float32)
    pr[order] = pr_sorted
    counts = np.bincount(lab, minlength=C)
    valid = (counts[lab] >= 2) & (counts[lab] <= B - 1)
    nv = int(valid.sum())
    loss = float((pr * valid).sum() / nv) if nv > 0 else 0.0
    return np.float32(loss), res


def kernel(embeddings, labels):
    loss, _ = run_cores(embeddings, labels, trace=False)
    return loss
